# revision 8
# baseline (speedup 1.0000x reference)
"""CrossMamba Trainium2 kernel.

Sharding: 8 cores = 4 batches x 2 scan directions. Core b and core 4+b
form a pair that works on batch b; both run the same SPMD program and
differ only in a 4-byte selector input (sel=1 fwd, sel=0 bwd).

Wall-clock is dominated by the axon tunnel (~60-100 MB/s), so the I/O
contract is tuned for bytes:
  - all weights are baked into the NEFF as inline Const tensors
    (transferred once at executable load, never per call)
  - each core uploads only HALF of its batch's ctx+q in fp16 (1.75 MB),
    natural time order; an on-device pair AllGather reassembles the
    full sequence on both cores of the pair
  - the backward direction is derived on device: exact 0/1 sel-blends
    choose operand placement, and negative-stride (reversed-AP) copies
    time-flip the data, so fwd and bwd cores run one program
  - the fwd+bwd results are summed on device with a pair AllReduce and
    each core downloads half the rows: 1 MB fp16 per core
  - the jitted executable is cached at module level, so repeat calls
    skip re-trace/re-compile; steady-state transfer is 14 MB up / 8 MB
    down per call

Per-core program:
  A0) bounce upload half to DRAM, pair AllGather -> full ctx/q
  A) x = blend(c_in(ctx)+segc, q+segq) with sel-driven placement/flip
  B) in_proj (u half) -> causal depthwise conv -> silu -> x_proj acc
  C) in_proj (z half) -> silu -> spill
  D) x_proj epilogue (dt / B / C rows)
  E) dt_proj -> softplus -> delta, dg = delta*u
  F) selective scan: per (channel-block, state): dA = exp(A_s*delta),
     dgB, hardware tensor_tensor_scan, C-readout, state accumulation;
     two passes of 8 states
  G) gate with silu(z), out_proj on the sel-chosen (and sel-flipped)
     time half, pair AllReduce, output the sel-chosen row half

GEMMs run in fp16 (f32 PSUM accumulate), scan math in f32/bf16.
"""
import hashlib
import numpy as np

B, Lq, Lc = 4, 1024, 1024
DQ, DC, DM = 1024, 768, 1024
DS, DCONV = 16, 4
DI, DTR = 2048, 64
L = Lc + Lq              # 2048
NCORE = 8
NE = DI // 128           # 16 u (or z) channel blocks
NK = DM // 128           # 8 k blocks for in_proj
NT = L // 512            # 4 time blocks of 512
NA = (DC + DM) // 128    # 14 row blocks in the packed activation half

_RUN = None              # cached (runner, weight fingerprint)
_WFP = None

_WKEYS = ("c_in_w", "seg_context", "seg_query", "in_proj_w", "conv_w",
          "conv_b", "x_proj_w", "dt_proj_w", "dt_proj_b", "A_log", "D",
          "mamba_out_w")


def _fingerprint(inputs):
    h = hashlib.blake2b(digest_size=16)
    for k in _WKEYS:
        a = np.ascontiguousarray(np.asarray(inputs[k]))
        h.update(k.encode())
        h.update(str(a.shape).encode())
        b = a.view(np.uint8).reshape(-1)
        step = max(1, b.size // 65536)
        h.update(bytes(b[::step][:65536]))
    return h.digest()


def _prep_weights(inputs):
    f32, f16 = np.float32, np.float16
    c_in_w = np.asarray(inputs["c_in_w"], f32)
    segc = np.asarray(inputs["seg_context"], f32).reshape(DM)
    segq = np.asarray(inputs["seg_query"], f32).reshape(DM)
    in_proj_w = np.asarray(inputs["in_proj_w"], f32)
    conv_w = np.asarray(inputs["conv_w"], f32)
    conv_b = np.asarray(inputs["conv_b"], f32)
    x_proj_w = np.asarray(inputs["x_proj_w"], f32)
    dt_proj_w = np.asarray(inputs["dt_proj_w"], f32)
    dt_proj_b = np.asarray(inputs["dt_proj_b"], f32)
    A = (-np.exp(np.asarray(inputs["A_log"], f32))).astype(f32)
    D = np.asarray(inputs["D"], f32)
    out_w = np.asarray(inputs["mamba_out_w"], f32)

    def blk(a, p=128):
        # [n*p, m] -> [p, n*m] with n-major free layout
        n = a.shape[0] // p
        return np.ascontiguousarray(
            a.reshape(n, p, -1).transpose(1, 0, 2).reshape(p, -1))

    return dict(
        Wc=blk(c_in_w.T).astype(f16),                     # [128, 6*1024]
        segc=np.ascontiguousarray(segc.reshape(NK, 128).T),   # [128, 8]
        segq=np.ascontiguousarray(segq.reshape(NK, 128).T),
        Win=np.ascontiguousarray(
            in_proj_w.reshape(32, 128, NK, 128).transpose(0, 3, 2, 1)
            .reshape(32, 128, NK * 128)).astype(f16),     # [32,128,1024]
        Wxp=blk(x_proj_w.T).astype(f16),                  # [128, 16*96]
        Wdt=np.ascontiguousarray(dt_proj_w.T).astype(f16),  # [64, 2048]
        Wout=np.ascontiguousarray(
            out_w.reshape(8, 128, NE, 128).transpose(3, 2, 0, 1)
            .reshape(128, NE * DM)).astype(f16),          # [128, 16*1024]
        convw=blk(conv_w),                                # [128, 16*4]
        convb=conv_b.reshape(NE, 128).T.copy(),
        dtb=dt_proj_b.reshape(NE, 128).T.copy(),
        Ah=blk(A),                                        # [128, 16*16]
        Dh=D.reshape(NE, 128).T.copy(),
    )


def _build(w):
    import concourse.bacc as bacc
    import concourse.tile as tile
    from concourse import mybir

    f32 = mybir.dt.float32
    f16 = mybir.dt.float16
    bf16 = mybir.dt.bfloat16
    MUL = mybir.AluOpType.mult
    ADD = mybir.AluOpType.add
    SUB = mybir.AluOpType.subtract
    BYP = mybir.AluOpType.bypass
    AF = mybir.ActivationFunctionType
    PAIRS = [[0, 4], [1, 5], [2, 6], [3, 7]]

    nc = bacc.Bacc("TRN2", target_bir_lowering=False, debug=False,
                   num_devices=NCORE)

    # ---- per-core external inputs ----
    # acth: this core's half of the batch's [ctx.T; q.T], natural time
    # order, fp16. Core b carries time cols 0:512, core 4+b cols 512:1024.
    acth_d = nc.dram_tensor("acth", [NA * 128, 512], f16,
                            kind="ExternalInput")
    sel_d = nc.dram_tensor("selv", [1, 1], f32, kind="ExternalInput")

    # ---- weights baked into the NEFF (loaded once, not per call) ----
    Wc_d = nc.inline_tensor(w["Wc"], name="Wc_i")
    segc_d = nc.inline_tensor(w["segc"], name="segc_i")
    segq_d = nc.inline_tensor(w["segq"], name="segq_i")
    Win_d = nc.inline_tensor(w["Win"], name="Win_i")
    Wxp_d = nc.inline_tensor(w["Wxp"], name="Wxp_i")
    Wdt_d = nc.inline_tensor(w["Wdt"], name="Wdt_i")
    Wout_d = nc.inline_tensor(w["Wout"], name="Wout_i")
    convw_d = nc.inline_tensor(w["convw"], name="convw_i")
    convb_d = nc.inline_tensor(w["convb"], name="convb_i")
    dtb_d = nc.inline_tensor(w["dtb"], name="dtb_i")
    Ah_d = nc.inline_tensor(w["Ah"], name="Ah_i")
    Dh_d = nc.inline_tensor(w["Dh"], name="Dh_i")

    # ---- DRAM scratch ----
    act_bnc = nc.dram_tensor("act_bnc", [NA * 128, 512], f16)
    ag_act = nc.dram_tensor("ag_act", [2, NA * 128, 512], f16)
    u_sp = nc.dram_tensor("u_sp", [DI, L], f16)
    zs_sp = nc.dram_tensor("zs_sp", [DI, L], bf16)
    dl_sp = nc.dram_tensor("dl_sp", [DI, L], f16)
    dg_sp = nc.dram_tensor("dg_sp", [DI, L], f16)
    bc_sp = nc.dram_tensor("bc_sp", [2 * DS, L], bf16)
    yacc_sp = nc.dram_tensor("yacc_sp", [DI, L], f32)
    yg_sp = nc.dram_tensor("yg_sp", [DI, L], f16)
    og_sp = nc.dram_tensor("og_sp", [DM, Lq], f16)
    og_sum = nc.dram_tensor("og_sum", [DM, Lq], f16)

    out_d = nc.dram_tensor("out", [DM // 2, Lq], f16, kind="ExternalOutput")

    with tile.TileContext(nc) as tc:
        with (
            tc.tile_pool(name="wp", bufs=1) as wp,
            tc.tile_pool(name="ps", bufs=3, space="PSUM") as ps,
        ):
            # ---------- phase A0: bounce + pair AllGather ----------
            with tc.tile_pool(name="p0", bufs=2) as p0:
                for rb in range(NA):
                    bt = p0.tile([128, 512], f16, tag="bnc")
                    nc.sync.dma_start(
                        bt[:], acth_d[rb * 128:(rb + 1) * 128, :])
                    nc.sync.dma_start(
                        act_bnc[rb * 128:(rb + 1) * 128, :], bt[:])
            nc.gpsimd.collective_compute(
                "AllGather", BYP, replica_groups=PAIRS,
                ins=[act_bnc[:].opt()], outs=[ag_act[:].opt()])

            # ---------- small persistent weights ----------
            convw = wp.tile([128, NE * DCONV], f32, tag="convw")
            nc.sync.dma_start(convw[:], convw_d[:])
            convb = wp.tile([128, NE], f32, tag="convb")
            nc.sync.dma_start(convb[:], convb_d[:])
            dtb = wp.tile([128, NE], f32, tag="dtb")
            nc.sync.dma_start(dtb[:], dtb_d[:])
            Ah = wp.tile([128, NE * DS], f32, tag="Ah")
            nc.sync.dma_start(Ah[:], Ah_d[:])
            Dh = wp.tile([128, NE], f32, tag="Dh")
            nc.sync.dma_start(Dh[:], Dh_d[:])
            Wxp = wp.tile([128, NE * 96], f16, tag="Wxp")
            nc.gpsimd.dma_start(Wxp[:], Wxp_d[:])
            Wdt = wp.tile([DTR, DI], f16, tag="Wdt")
            nc.gpsimd.dma_start(Wdt[:], Wdt_d[:])
            dt_r = wp.tile([DTR, L], f16, tag="dt_r")
            sel = wp.tile([128, 1], f32, tag="sel")
            nc.sync.dma_start(sel[:], sel_d[0:1, :].partition_broadcast(128))

            with tc.tile_pool(name="px", bufs=1) as px:
                # full-sequence x, fp16, 32 KB/part; lives phases A-C
                x_r = [px.tile([128, L], f16, tag=f"x{db}", name=f"x{db}")
                       for db in range(NK)]

                # ---------- phase A ----------
                with tc.tile_pool(name="pa", bufs=1) as pa:
                    Wc = pa.tile([128, 6 * DM], f16, tag="Wc")
                    nc.gpsimd.dma_start(Wc[:], Wc_d[:])
                    segc = pa.tile([128, NK], f32, tag="segc")
                    nc.sync.dma_start(segc[:], segc_d[:])
                    segq = pa.tile([128, NK], f32, tag="segq")
                    nc.sync.dma_start(segq[:], segq_d[:])
                    ctx_sb = []
                    for kb in range(6):
                        t0 = pa.tile([128, Lc], f16, tag=f"ctxa{kb}",
                                     name=f"ctxa{kb}")
                        for hf in range(2):
                            nc.gpsimd.dma_start(
                                t0[:, hf * 512:(hf + 1) * 512],
                                ag_act[hf, kb * 128:(kb + 1) * 128, :])
                        ctx_sb.append(t0)
                    for db in range(NK):
                        qt = pa.tile([128, Lq], f16, tag="qt", bufs=2)
                        for hf in range(2):
                            nc.sync.dma_start(
                                qt[:, hf * 512:(hf + 1) * 512],
                                ag_act[hf, DC + db * 128:
                                       DC + (db + 1) * 128, :])
                        cparts, qparts = [], []
                        for j in range(2):
                            jl = j * 512
                            acc = ps.tile([128, 512], f32, tag="pp")
                            for kb in range(6):
                                nc.tensor.matmul(
                                    acc[:],
                                    Wc[:, kb * DM + db * 128:
                                       kb * DM + (db + 1) * 128],
                                    ctx_sb[kb][:, jl:jl + 512],
                                    start=(kb == 0), stop=(kb == 5))
                            cp = pa.tile([128, 512], f32, tag=f"cpart{j}",
                                         name=f"cpart{j}", bufs=2)
                            nc.vector.tensor_scalar(
                                out=cp[:], in0=acc[:],
                                scalar1=segc[:, db:db + 1], scalar2=None,
                                op0=ADD)
                            qp = pa.tile([128, 512], f32, tag=f"qpart{j}",
                                         name=f"qpart{j}", bufs=2)
                            nc.vector.tensor_scalar(
                                out=qp[:], in0=qt[:, jl:jl + 512],
                                scalar1=segq[:, db:db + 1], scalar2=None,
                                op0=ADD)
                            cparts.append(cp)
                            qparts.append(qp)
                        for j in range(2):
                            jl = j * 512
                            # bwd (sel=0) wants time-flipped q in half0 and
                            # time-flipped c in half1: block 1-j reversed
                            crev = pa.tile([128, 512], f32, tag="crev",
                                           bufs=2)
                            nc.scalar.copy(crev[:], cparts[1 - j][:, ::-1])
                            qrev = pa.tile([128, 512], f32, tag="qrev",
                                           bufs=2)
                            nc.scalar.copy(qrev[:], qparts[1 - j][:, ::-1])
                            d0 = pa.tile([128, 512], f32, tag="d0", bufs=2)
                            nc.vector.tensor_tensor(
                                out=d0[:], in0=cparts[j][:], in1=qrev[:],
                                op=SUB)
                            s0 = pa.tile([128, 512], f32, tag="s0", bufs=2)
                            nc.vector.tensor_scalar(
                                out=s0[:], in0=d0[:], scalar1=sel[:, 0:1],
                                scalar2=None, op0=MUL)
                            nc.vector.tensor_tensor(
                                out=x_r[db][:, jl:jl + 512],
                                in0=qrev[:], in1=s0[:], op=ADD)
                            d1 = pa.tile([128, 512], f32, tag="d1", bufs=2)
                            nc.vector.tensor_tensor(
                                out=d1[:], in0=qparts[j][:], in1=crev[:],
                                op=SUB)
                            s1 = pa.tile([128, 512], f32, tag="s1", bufs=2)
                            nc.vector.tensor_scalar(
                                out=s1[:], in0=d1[:], scalar1=sel[:, 0:1],
                                scalar2=None, op0=MUL)
                            nc.vector.tensor_tensor(
                                out=x_r[db][:, Lc + jl:Lc + jl + 512],
                                in0=crev[:], in1=s1[:], op=ADD)

                # ---------- phases B/C/D ----------
                with (tc.tile_pool(name="pb", bufs=1) as pb,
                      tc.tile_pool(name="psxp", bufs=1, space="PSUM") as psxp):
                    xp_acc = [psxp.tile([96, 512], f32, tag=f"xp{tb}",
                                        name=f"xp{tb}") for tb in range(NT)]
                    for e in range(NE):
                        wt = pb.tile([128, NK * 128], f16, tag="winstream",
                                     bufs=2)
                        nc.gpsimd.dma_start(wt[:], Win_d[e, :, :])
                        upre = pb.tile([128, L + 3], f32, tag="upre", bufs=2)
                        nc.gpsimd.memset(upre[:, 0:3], 0.0)
                        for tb in range(NT):
                            acc = ps.tile([128, 512], f32, tag="pp")
                            for kb in range(NK):
                                nc.tensor.matmul(
                                    acc[:], wt[:, kb * 128:(kb + 1) * 128],
                                    x_r[kb][:, tb * 512:(tb + 1) * 512],
                                    start=(kb == 0), stop=(kb == NK - 1))
                            nc.scalar.copy(
                                upre[:, 3 + tb * 512: 3 + (tb + 1) * 512],
                                acc[:])
                        # causal depthwise conv: taps read aligned slices
                        cacc = pb.tile([128, L], f32, tag="cacc0", bufs=2)
                        nc.vector.tensor_scalar(
                            out=cacc[:], in0=upre[:, 0:L],
                            scalar1=convw[:, e * DCONV: e * DCONV + 1],
                            scalar2=None, op0=MUL)
                        for k in (1, 2, 3):
                            nxt = pb.tile([128, L], f32, tag=f"cacc{k % 2}",
                                          name=f"cacc_{k}", bufs=2)
                            nc.vector.scalar_tensor_tensor(
                                out=nxt[:], in0=upre[:, k:k + L],
                                scalar=convw[:, e * DCONV + k:
                                             e * DCONV + k + 1],
                                in1=cacc[:], op0=MUL, op1=ADD)
                            cacc = nxt
                        usilu = pb.tile([128, L], f16, tag="usilu", bufs=2)
                        nc.scalar.activation(usilu[:], cacc[:], AF.Silu,
                                             bias=convb[:, e:e + 1])
                        nc.gpsimd.dma_start(
                            u_sp[e * 128:(e + 1) * 128, :], usilu[:])
                        for tb in range(NT):
                            nc.tensor.matmul(
                                xp_acc[tb][:],
                                Wxp[:, e * 96:(e + 1) * 96],
                                usilu[:, tb * 512:(tb + 1) * 512],
                                start=(e == 0), stop=(e == NE - 1))

                    # phase C: z half -> silu -> spill
                    for e in range(NE):
                        wt = pb.tile([128, NK * 128], f16, tag="winstream",
                                     name="wtz", bufs=2)
                        nc.gpsimd.dma_start(wt[:], Win_d[NE + e, :, :])
                        for tb in range(NT):
                            acc = ps.tile([128, 512], f32, tag="pp")
                            for kb in range(NK):
                                nc.tensor.matmul(
                                    acc[:], wt[:, kb * 128:(kb + 1) * 128],
                                    x_r[kb][:, tb * 512:(tb + 1) * 512],
                                    start=(kb == 0), stop=(kb == NK - 1))
                            zt = pb.tile([128, 512], bf16, tag="zt", bufs=2)
                            nc.scalar.activation(zt[:], acc[:], AF.Silu)
                            nc.sync.dma_start(
                                zs_sp[e * 128:(e + 1) * 128,
                                      tb * 512:(tb + 1) * 512], zt[:])

                    # phase D: x_proj epilogue
                    for tb in range(NT):
                        nc.scalar.copy(dt_r[:, tb * 512:(tb + 1) * 512],
                                       xp_acc[tb][0:DTR, :])
                        bct = pb.tile([2 * DS, 512], bf16, tag="bct", bufs=2)
                        nc.scalar.copy(bct[:], xp_acc[tb][DTR:96, :])
                        nc.sync.dma_start(
                            bc_sp[:, tb * 512:(tb + 1) * 512], bct[:])

            # ---------- phase E: dt_proj -> delta, dg ----------
            with tc.tile_pool(name="pe", bufs=1) as pe:
                for e in range(NE):
                    delta = pe.tile([128, L], f32, tag="delta", bufs=2)
                    for tb in range(NT):
                        acc = ps.tile([128, 512], f32, tag="pp")
                        nc.tensor.matmul(
                            acc[:], Wdt[:, e * 128:(e + 1) * 128],
                            dt_r[:, tb * 512:(tb + 1) * 512],
                            start=True, stop=True)
                        # softplus(x + b) = ln(1 + exp(x + b)); inputs here
                        # are small (|x|<6) so exp cannot overflow
                        ex = pe.tile([128, 512], f32, tag="spexp", bufs=2)
                        nc.scalar.activation(
                            ex[:], acc[:], AF.Exp, bias=dtb[:, e:e + 1])
                        nc.scalar.activation(
                            delta[:, tb * 512:(tb + 1) * 512], ex[:],
                            AF.Ln, bias=1.0)
                    nc.gpsimd.dma_start(
                        dl_sp[e * 128:(e + 1) * 128, :], delta[:])
                    ub = pe.tile([128, L], f16, tag="ub_e", bufs=2)
                    nc.sync.dma_start(ub[:], u_sp[e * 128:(e + 1) * 128, :])
                    dg = pe.tile([128, L], f16, tag="dg_e", bufs=2)
                    nc.vector.tensor_tensor(out=dg[:], in0=delta[:],
                                            in1=ub[:], op=MUL)
                    nc.sync.dma_start(
                        dg_sp[e * 128:(e + 1) * 128, :], dg[:])

            # ---------- phase F: selective scan ----------
            with tc.tile_pool(name="pf", bufs=1) as pf:
                for p in range(2):
                    Bb, Cb = [], []
                    for si in range(8):
                        s = p * 8 + si
                        bb = pf.tile([128, L], bf16, tag=f"Bb{si}",
                                     name=f"Bb{si}")
                        nc.sync.dma_start(
                            bb[:], bc_sp[s:s + 1, :].partition_broadcast(128))
                        cb = pf.tile([128, L], bf16, tag=f"Cb{si}",
                                     name=f"Cb{si}")
                        nc.sync.dma_start(
                            cb[:],
                            bc_sp[DS + s:DS + s + 1, :].partition_broadcast(128))
                        Bb.append(bb)
                        Cb.append(cb)
                    for e in range(NE):
                        dl = pf.tile([128, L], f16, tag="dl_f", bufs=2)
                        nc.sync.dma_start(
                            dl[:], dl_sp[e * 128:(e + 1) * 128, :])
                        dgt = pf.tile([128, L], f16, tag="dg_f", bufs=2)
                        nc.sync.dma_start(
                            dgt[:], dg_sp[e * 128:(e + 1) * 128, :])
                        if p == 0:
                            ub = pf.tile([128, L], f16, tag="ub_f", bufs=2)
                            nc.sync.dma_start(
                                ub[:], u_sp[e * 128:(e + 1) * 128, :])
                            yacc = pf.tile([128, L], f32, tag="yacc0",
                                           name="yacc_i", bufs=1)
                            nc.vector.tensor_scalar(
                                out=yacc[:], in0=ub[:],
                                scalar1=Dh[:, e:e + 1], scalar2=None, op0=MUL)
                        else:
                            yacc = pf.tile([128, L], f32, tag="yacc0",
                                           name="yacc_l", bufs=1)
                            nc.sync.dma_start(
                                yacc[:], yacc_sp[e * 128:(e + 1) * 128, :])
                        for si in range(8):
                            s = p * 8 + si
                            dA = pf.tile([128, L], f32, tag="dA", bufs=2)
                            nc.scalar.activation(
                                dA[:], dl[:], AF.Exp,
                                scale=Ah[:, e * DS + s: e * DS + s + 1])
                            dgB = pf.tile([128, L], bf16, tag="dgB", bufs=2)
                            nc.vector.tensor_tensor(
                                out=dgB[:], in0=dgt[:], in1=Bb[si][:], op=MUL)
                            h = pf.tile([128, L], bf16, tag="h", bufs=2)
                            nc.vector.tensor_tensor_scan(
                                h[:], dA[:], dgB[:], 0.0, op0=MUL, op1=ADD)
                            ch = pf.tile([128, L], bf16, tag="ch", bufs=2)
                            nc.vector.tensor_tensor(
                                out=ch[:], in0=h[:], in1=Cb[si][:], op=MUL)
                            ynew = pf.tile([128, L], f32,
                                           tag=f"yacc{(si + 1) % 2}",
                                           name=f"yacc_{si}", bufs=1)
                            nc.gpsimd.tensor_tensor(
                                out=ynew[:], in0=yacc[:], in1=ch[:], op=ADD)
                            yacc = ynew
                        if p == 0:
                            nc.sync.dma_start(
                                yacc_sp[e * 128:(e + 1) * 128, :], yacc[:])
                        else:
                            zst = pf.tile([128, L], bf16, tag="zs_f", bufs=2)
                            nc.sync.dma_start(
                                zst[:], zs_sp[e * 128:(e + 1) * 128, :])
                            yg = pf.tile([128, L], f16, tag="yg", bufs=2)
                            nc.vector.tensor_tensor(
                                out=yg[:], in0=yacc[:], in1=zst[:], op=MUL)
                            nc.sync.dma_start(
                                yg_sp[e * 128:(e + 1) * 128, :], yg[:])

            # ---------- phase G: out_proj on the sel-chosen half ----------
            with tc.tile_pool(name="pg", bufs=1) as pg:
                Wout = pg.tile([128, NE * DM], f16, tag="Wout")
                nc.gpsimd.dma_start(Wout[:], Wout_d[:])
                for j in range(2):
                    jl = j * 512
                    ysel = []
                    for kb in range(NE):
                        # fwd (sel=1): natural cols Lc+jl..; bwd (sel=0):
                        # cols (1-j)*512.. time-reversed
                        ylo = pg.tile([128, 512], f16, tag="ylo", bufs=2)
                        nc.sync.dma_start(
                            ylo[:], yg_sp[kb * 128:(kb + 1) * 128,
                                          (1 - j) * 512:(2 - j) * 512])
                        yhi = pg.tile([128, 512], f16, tag="yhi", bufs=2)
                        nc.sync.dma_start(
                            yhi[:], yg_sp[kb * 128:(kb + 1) * 128,
                                          Lc + jl:Lc + jl + 512])
                        yrev = pg.tile([128, 512], f16, tag="yrev", bufs=2)
                        nc.scalar.copy(yrev[:], ylo[:, ::-1])
                        dft = pg.tile([128, 512], f32, tag="dft", bufs=2)
                        nc.vector.tensor_tensor(
                            out=dft[:], in0=yhi[:], in1=yrev[:], op=SUB)
                        sdf = pg.tile([128, 512], f32, tag="sdf", bufs=2)
                        nc.vector.tensor_scalar(
                            out=sdf[:], in0=dft[:], scalar1=sel[:, 0:1],
                            scalar2=None, op0=MUL)
                        ys = pg.tile([128, 512], f16, tag=f"ys{kb}",
                                     name=f"ys{kb}", bufs=1)
                        nc.vector.tensor_tensor(
                            out=ys[:], in0=yrev[:], in1=sdf[:], op=ADD)
                        ysel.append(ys)
                    for mb in range(8):
                        acc = ps.tile([128, 512], f32, tag="pp")
                        for kb in range(NE):
                            nc.tensor.matmul(
                                acc[:],
                                Wout[:, kb * DM + mb * 128:
                                     kb * DM + (mb + 1) * 128],
                                ysel[kb][:], start=(kb == 0),
                                stop=(kb == NE - 1))
                        ot = pg.tile([128, 512], f16, tag="ot", bufs=2)
                        nc.scalar.copy(ot[:], acc[:])
                        nc.sync.dma_start(
                            og_sp[mb * 128:(mb + 1) * 128, jl:jl + 512],
                            ot[:])

                # pair AllReduce: fwd+bwd sum lands on both cores
                nc.gpsimd.collective_compute(
                    "AllReduce", ADD, replica_groups=PAIRS,
                    ins=[og_sp[:].opt()], outs=[og_sum[:].opt()])

                # each core outputs its row half: fwd rows 0:512, bwd 512:1024
                for bb in range(4):
                    for j in range(2):
                        jl = j * 512
                        stp = pg.tile([128, 512], f16, tag="stp", bufs=2)
                        nc.sync.dma_start(
                            stp[:], og_sum[bb * 128:(bb + 1) * 128,
                                           jl:jl + 512])
                        sbt = pg.tile([128, 512], f16, tag="sbt", bufs=2)
                        nc.sync.dma_start(
                            sbt[:], og_sum[(4 + bb) * 128:(5 + bb) * 128,
                                           jl:jl + 512])
                        dfo = pg.tile([128, 512], f32, tag="dfo", bufs=2)
                        nc.vector.tensor_tensor(
                            out=dfo[:], in0=stp[:], in1=sbt[:], op=SUB)
                        sfo = pg.tile([128, 512], f32, tag="sfo", bufs=2)
                        nc.vector.tensor_scalar(
                            out=sfo[:], in0=dfo[:], scalar1=sel[:, 0:1],
                            scalar2=None, op0=MUL)
                        oh = pg.tile([128, 512], f16, tag="oh", bufs=2)
                        nc.vector.tensor_tensor(
                            out=oh[:], in0=sbt[:], in1=sfo[:], op=ADD)
                        nc.sync.dma_start(
                            out_d[bb * 128:(bb + 1) * 128, jl:jl + 512],
                            oh[:])

    nc.compile()
    return nc


def _make_runner(nc):
    """Jit the SPMD dispatch once; repeat calls hit the C++ fast path.

    Mirrors what bass_utils.run_bass_kernel_spmd does under axon
    (bass2jax.run_bass_via_pjrt), minus the per-call re-jit and the
    donated zero output buffers (the kernel writes every output
    element, so uninitialized outputs are fine).
    """
    import jax
    import numpy as np
    from jax.sharding import Mesh, PartitionSpec
    from jax.experimental.shard_map import shard_map
    from concourse import mybir
    from concourse.bass2jax import (_bass_exec_p, install_neuronx_cc_hook,
                                    partition_id_tensor)

    install_neuronx_cc_hook()
    partition_name = (nc.partition_id_tensor.name
                      if nc.partition_id_tensor else None)
    in_names, out_names, out_avals = [], [], []
    for alloc in nc.m.functions[0].allocations:
        if not isinstance(alloc, mybir.MemoryLocationSet):
            continue
        name = alloc.memorylocations[0].name
        if alloc.kind == "ExternalInput":
            if name != partition_name:
                in_names.append(name)
        elif alloc.kind == "ExternalOutput":
            out_names.append(name)
            out_avals.append(jax.core.ShapedArray(
                tuple(alloc.tensor_shape), mybir.dt.np(alloc.dtype)))
    in_names_all = list(in_names)
    if partition_name is not None:
        in_names_all.append(partition_name)

    def _body(*args):
        operands = list(args)
        if partition_name is not None:
            operands.append(partition_id_tensor())
        return tuple(_bass_exec_p.bind(
            *operands, out_avals=tuple(out_avals),
            in_names=tuple(in_names_all), out_names=tuple(out_names),
            lowering_input_output_aliases=(),
            sim_require_finite=True, sim_require_nnan=True, nc=nc))

    devices = jax.devices()[:NCORE]
    mesh = Mesh(np.asarray(devices), ("core",))
    n_in = len(in_names)
    sharded = jax.jit(shard_map(
        _body, mesh=mesh, in_specs=(PartitionSpec("core"),) * n_in,
        out_specs=(PartitionSpec("core"),) * len(out_names),
        check_rep=False))

    def run(per_name_globals):
        args = [per_name_globals[name] for name in in_names]
        outs = sharded(*args)
        return {name: outs[i] for i, name in enumerate(out_names)}

    return run


def kernel(**inputs) -> np.ndarray:
    global _RUN, _WFP
    fp = _fingerprint(inputs)
    if _RUN is None or fp != _WFP:
        _RUN = _make_runner(_build(_prep_weights(inputs)))
        _WFP = fp

    q = np.asarray(inputs["query"], np.float32)
    ctx = np.asarray(inputs["context"], np.float32)
    # transpose once in fp16 (contiguous passes), then slice halves
    cT16 = np.ascontiguousarray(
        ctx.astype(np.float16).transpose(0, 2, 1))    # [B, DC, Lc]
    qT16 = np.ascontiguousarray(
        q.astype(np.float16).transpose(0, 2, 1))      # [B, DM, Lq]
    gact = np.empty((NCORE, NA * 128, 512), np.float16)
    for b in range(B):
        for hf in range(2):
            hs = slice(hf * 512, (hf + 1) * 512)
            core = b + 4 * hf
            gact[core, :DC] = cT16[b, :, hs]
            gact[core, DC:] = qT16[b, :, hs]
    gsel = np.zeros((NCORE, 1), np.float32)
    gsel[:4] = 1.0

    res = _RUN({"acth": gact.reshape(NCORE * NA * 128, 512), "selv": gsel})
    out = res["out"]                      # jax array, global [8*512, Lq]
    shards = sorted(out.addressable_shards,
                    key=lambda s: s.index[0].start or 0)
    for s in shards:
        s.data.copy_to_host_async()
    # pipeline: fetch shards in order, combine batch b once 4+b arrives
    y = np.empty((B, Lq, DM), np.float32)
    half = np.empty((DM, Lq), np.float32)
    parts = [None] * NCORE
    for i in range(NCORE):
        parts[i] = np.asarray(shards[i].data)
        if i >= 4:
            b = i - 4
            half[:DM // 2] = parts[b]
            half[DM // 2:] = parts[i]
            np.multiply(half.T, 0.5, out=y[b])
    return y


# revision 10
# speedup vs baseline: 1.1917x; 1.1917x over previous
"""CrossMamba Trainium2 kernel.

Sharding: 8 cores = 4 batches x 2 scan directions. Core b and core 4+b
form a pair that works on batch b; both run the same SPMD program and
differ only in a 4-byte selector input (sel=1 fwd, sel=0 bwd).

Wall-clock is dominated by the axon tunnel (~60-100 MB/s), so the I/O
contract is tuned for bytes:
  - all weights are baked into the NEFF as inline Const tensors
    (transferred once at executable load, never per call)
  - each core uploads only HALF of its batch's ctx+q in fp16 (1.75 MB),
    natural time order; an on-device pair AllGather reassembles the
    full sequence on both cores of the pair
  - the backward direction is derived on device: exact 0/1 sel-blends
    choose operand placement, and negative-stride (reversed-AP) copies
    time-flip the data, so fwd and bwd cores run one program
  - the fwd+bwd results are summed on device with a pair AllReduce and
    each core downloads half the rows: 1 MB fp16 per core
  - the jitted executable is cached at module level, so repeat calls
    skip re-trace/re-compile; steady-state transfer is 14 MB up / 8 MB
    down per call

Per-core program:
  A0) bounce upload half to DRAM, pair AllGather -> full ctx/q
  A) x = blend(c_in(ctx)+segc, q+segq) with sel-driven placement/flip
  B) in_proj (u half) -> causal depthwise conv -> silu -> x_proj acc
  C) in_proj (z half) -> silu -> spill
  D) x_proj epilogue (dt / B / C rows)
  E) dt_proj -> softplus -> delta, dg = delta*u
  F) selective scan: per (channel-block, state): dA = exp(A_s*delta),
     dgB, hardware tensor_tensor_scan, C-readout, state accumulation;
     two passes of 8 states
  G) gate with silu(z), out_proj on the sel-chosen (and sel-flipped)
     time half, pair AllReduce, output the sel-chosen row half

GEMMs run in fp16 (f32 PSUM accumulate), scan math in f32/bf16.
"""
import hashlib
import numpy as np

B, Lq, Lc = 4, 1024, 1024
DQ, DC, DM = 1024, 768, 1024
DS, DCONV = 16, 4
DI, DTR = 2048, 64
L = Lc + Lq              # 2048
NCORE = 8
NE = DI // 128           # 16 u (or z) channel blocks
NK = DM // 128           # 8 k blocks for in_proj
NT = L // 512            # 4 time blocks of 512
NA = (DC + DM) // 128    # 14 row blocks in the packed activation half

_RUN = None              # cached (runner, weight fingerprint)
_WFP = None

_WKEYS = ("c_in_w", "seg_context", "seg_query", "in_proj_w", "conv_w",
          "conv_b", "x_proj_w", "dt_proj_w", "dt_proj_b", "A_log", "D",
          "mamba_out_w")


def _fingerprint(inputs):
    h = hashlib.blake2b(digest_size=16)
    for k in _WKEYS:
        a = np.ascontiguousarray(np.asarray(inputs[k]))
        h.update(k.encode())
        h.update(str(a.shape).encode())
        b = a.view(np.uint8).reshape(-1)
        step = max(1, b.size // 65536)
        h.update(bytes(b[::step][:65536]))
    return h.digest()


def _prep_weights(inputs):
    f32, f16 = np.float32, np.float16
    c_in_w = np.asarray(inputs["c_in_w"], f32)
    segc = np.asarray(inputs["seg_context"], f32).reshape(DM)
    segq = np.asarray(inputs["seg_query"], f32).reshape(DM)
    in_proj_w = np.asarray(inputs["in_proj_w"], f32)
    conv_w = np.asarray(inputs["conv_w"], f32)
    conv_b = np.asarray(inputs["conv_b"], f32)
    x_proj_w = np.asarray(inputs["x_proj_w"], f32)
    dt_proj_w = np.asarray(inputs["dt_proj_w"], f32)
    dt_proj_b = np.asarray(inputs["dt_proj_b"], f32)
    A = (-np.exp(np.asarray(inputs["A_log"], f32))).astype(f32)
    D = np.asarray(inputs["D"], f32)
    out_w = np.asarray(inputs["mamba_out_w"], f32)

    def blk(a, p=128):
        # [n*p, m] -> [p, n*m] with n-major free layout
        n = a.shape[0] // p
        return np.ascontiguousarray(
            a.reshape(n, p, -1).transpose(1, 0, 2).reshape(p, -1))

    return dict(
        Wc=blk(c_in_w.T).astype(f16),                     # [128, 6*1024]
        segc=np.ascontiguousarray(segc.reshape(NK, 128).T),   # [128, 8]
        segq=np.ascontiguousarray(segq.reshape(NK, 128).T),
        Win=np.ascontiguousarray(
            in_proj_w.reshape(32, 128, NK, 128).transpose(0, 3, 2, 1)
            .reshape(32, 128, NK * 128)).astype(f16),     # [32,128,1024]
        Wxp=blk(x_proj_w.T).astype(f16),                  # [128, 16*96]
        Wdt=np.ascontiguousarray(dt_proj_w.T).astype(f16),  # [64, 2048]
        Wout=np.ascontiguousarray(
            out_w.reshape(8, 128, NE, 128).transpose(3, 2, 0, 1)
            .reshape(128, NE * DM)).astype(f16),          # [128, 16*1024]
        convw=blk(conv_w),                                # [128, 16*4]
        convb=conv_b.reshape(NE, 128).T.copy(),
        dtb=dt_proj_b.reshape(NE, 128).T.copy(),
        Ah=blk(A),                                        # [128, 16*16]
        Dh=D.reshape(NE, 128).T.copy(),
    )


def _build(w):
    import concourse.bacc as bacc
    import concourse.tile as tile
    from concourse import mybir

    f32 = mybir.dt.float32
    f16 = mybir.dt.float16
    bf16 = mybir.dt.bfloat16
    MUL = mybir.AluOpType.mult
    ADD = mybir.AluOpType.add
    SUB = mybir.AluOpType.subtract
    BYP = mybir.AluOpType.bypass
    AF = mybir.ActivationFunctionType
    PAIRS = [[0, 4], [1, 5], [2, 6], [3, 7]]

    nc = bacc.Bacc("TRN2", target_bir_lowering=False, debug=False,
                   num_devices=NCORE)

    # ---- per-core external inputs ----
    # acth: this core's half of the batch's [ctx.T; q.T], natural time
    # order, fp16. Core b carries time cols 0:512, core 4+b cols 512:1024.
    acth_d = nc.dram_tensor("acth", [NA * 128, 512], f16,
                            kind="ExternalInput")
    sel_d = nc.dram_tensor("selv", [1, 1], f32, kind="ExternalInput")

    # ---- weights baked into the NEFF (loaded once, not per call) ----
    Wc_d = nc.inline_tensor(w["Wc"], name="Wc_i")
    segc_d = nc.inline_tensor(w["segc"], name="segc_i")
    segq_d = nc.inline_tensor(w["segq"], name="segq_i")
    Win_d = nc.inline_tensor(w["Win"], name="Win_i")
    Wxp_d = nc.inline_tensor(w["Wxp"], name="Wxp_i")
    Wdt_d = nc.inline_tensor(w["Wdt"], name="Wdt_i")
    Wout_d = nc.inline_tensor(w["Wout"], name="Wout_i")
    convw_d = nc.inline_tensor(w["convw"], name="convw_i")
    convb_d = nc.inline_tensor(w["convb"], name="convb_i")
    dtb_d = nc.inline_tensor(w["dtb"], name="dtb_i")
    Ah_d = nc.inline_tensor(w["Ah"], name="Ah_i")
    Dh_d = nc.inline_tensor(w["Dh"], name="Dh_i")

    # ---- DRAM scratch ----
    act_bnc = nc.dram_tensor("act_bnc", [NA * 128, 512], f16)
    ag_act = nc.dram_tensor("ag_act", [2, NA * 128, 512], f16)
    u_sp = nc.dram_tensor("u_sp", [DI, L], f16)
    zs_sp = nc.dram_tensor("zs_sp", [DI, L], bf16)
    dl_sp = nc.dram_tensor("dl_sp", [DI, L], f16)
    dg_sp = nc.dram_tensor("dg_sp", [DI, L], f16)
    bc_sp = nc.dram_tensor("bc_sp", [2 * DS, L], bf16)
    yacc_sp = nc.dram_tensor("yacc_sp", [DI, L], f32)
    yg_sp = nc.dram_tensor("yg_sp", [DI, L], f16)
    og_sp = nc.dram_tensor("og_sp", [DM, Lq], f16)
    og_sum = nc.dram_tensor("og_sum", [DM, Lq], f16)

    out_d = nc.dram_tensor("out", [DM // 2, Lq], f16, kind="ExternalOutput")

    with tile.TileContext(nc) as tc:
        with (
            tc.tile_pool(name="wp", bufs=1) as wp,
            tc.tile_pool(name="ps", bufs=3, space="PSUM") as ps,
        ):
            # ---------- phase A0: bounce + pair AllGather ----------
            with tc.tile_pool(name="p0", bufs=2) as p0:
                for rb in range(NA):
                    bt = p0.tile([128, 512], f16, tag="bnc")
                    nc.sync.dma_start(
                        bt[:], acth_d[rb * 128:(rb + 1) * 128, :])
                    nc.sync.dma_start(
                        act_bnc[rb * 128:(rb + 1) * 128, :], bt[:])
            nc.gpsimd.collective_compute(
                "AllGather", BYP, replica_groups=PAIRS,
                ins=[act_bnc[:].opt()], outs=[ag_act[:].opt()])

            # ---------- small persistent weights ----------
            convw = wp.tile([128, NE * DCONV], f32, tag="convw")
            nc.sync.dma_start(convw[:], convw_d[:])
            convb = wp.tile([128, NE], f32, tag="convb")
            nc.sync.dma_start(convb[:], convb_d[:])
            dtb = wp.tile([128, NE], f32, tag="dtb")
            nc.sync.dma_start(dtb[:], dtb_d[:])
            Ah = wp.tile([128, NE * DS], f32, tag="Ah")
            nc.sync.dma_start(Ah[:], Ah_d[:])
            Dh = wp.tile([128, NE], f32, tag="Dh")
            nc.sync.dma_start(Dh[:], Dh_d[:])
            Wxp = wp.tile([128, NE * 96], f16, tag="Wxp")
            nc.gpsimd.dma_start(Wxp[:], Wxp_d[:])
            Wdt = wp.tile([DTR, DI], f16, tag="Wdt")
            nc.gpsimd.dma_start(Wdt[:], Wdt_d[:])
            dt_r = wp.tile([DTR, L], f16, tag="dt_r")
            sel = wp.tile([128, 1], f32, tag="sel")
            nc.sync.dma_start(sel[:], sel_d[0:1, :].partition_broadcast(128))

            with tc.tile_pool(name="px", bufs=1) as px:
                # full-sequence x, fp16, 32 KB/part; lives phases A-C
                x_r = [px.tile([128, L], f16, tag=f"x{db}", name=f"x{db}")
                       for db in range(NK)]

                # ---------- phase A ----------
                with tc.tile_pool(name="pa", bufs=1) as pa:
                    Wc = pa.tile([128, 6 * DM], f16, tag="Wc")
                    nc.gpsimd.dma_start(Wc[:], Wc_d[:])
                    segc = pa.tile([128, NK], f32, tag="segc")
                    nc.sync.dma_start(segc[:], segc_d[:])
                    segq = pa.tile([128, NK], f32, tag="segq")
                    nc.sync.dma_start(segq[:], segq_d[:])
                    ctx_sb = []
                    for kb in range(6):
                        t0 = pa.tile([128, Lc], f16, tag=f"ctxa{kb}",
                                     name=f"ctxa{kb}")
                        for hf in range(2):
                            nc.gpsimd.dma_start(
                                t0[:, hf * 512:(hf + 1) * 512],
                                ag_act[hf, kb * 128:(kb + 1) * 128, :])
                        ctx_sb.append(t0)
                    for db in range(NK):
                        qt = pa.tile([128, Lq], f16, tag="qt", bufs=2)
                        for hf in range(2):
                            nc.sync.dma_start(
                                qt[:, hf * 512:(hf + 1) * 512],
                                ag_act[hf, DC + db * 128:
                                       DC + (db + 1) * 128, :])
                        cparts, qparts = [], []
                        for j in range(2):
                            jl = j * 512
                            acc = ps.tile([128, 512], f32, tag="pp")
                            for kb in range(6):
                                nc.tensor.matmul(
                                    acc[:],
                                    Wc[:, kb * DM + db * 128:
                                       kb * DM + (db + 1) * 128],
                                    ctx_sb[kb][:, jl:jl + 512],
                                    start=(kb == 0), stop=(kb == 5))
                            cp = pa.tile([128, 512], f32, tag=f"cpart{j}",
                                         name=f"cpart{j}", bufs=2)
                            nc.vector.tensor_scalar(
                                out=cp[:], in0=acc[:],
                                scalar1=segc[:, db:db + 1], scalar2=None,
                                op0=ADD)
                            qp = pa.tile([128, 512], f32, tag=f"qpart{j}",
                                         name=f"qpart{j}", bufs=2)
                            nc.vector.tensor_scalar(
                                out=qp[:], in0=qt[:, jl:jl + 512],
                                scalar1=segq[:, db:db + 1], scalar2=None,
                                op0=ADD)
                            cparts.append(cp)
                            qparts.append(qp)
                        for j in range(2):
                            jl = j * 512
                            # bwd (sel=0) wants time-flipped q in half0 and
                            # time-flipped c in half1: block 1-j reversed
                            crev = pa.tile([128, 512], f32, tag="crev",
                                           bufs=2)
                            nc.scalar.copy(crev[:], cparts[1 - j][:, ::-1])
                            qrev = pa.tile([128, 512], f32, tag="qrev",
                                           bufs=2)
                            nc.scalar.copy(qrev[:], qparts[1 - j][:, ::-1])
                            d0 = pa.tile([128, 512], f32, tag="d0", bufs=2)
                            nc.vector.tensor_tensor(
                                out=d0[:], in0=cparts[j][:], in1=qrev[:],
                                op=SUB)
                            s0 = pa.tile([128, 512], f32, tag="s0", bufs=2)
                            nc.vector.tensor_scalar(
                                out=s0[:], in0=d0[:], scalar1=sel[:, 0:1],
                                scalar2=None, op0=MUL)
                            nc.vector.tensor_tensor(
                                out=x_r[db][:, jl:jl + 512],
                                in0=qrev[:], in1=s0[:], op=ADD)
                            d1 = pa.tile([128, 512], f32, tag="d1", bufs=2)
                            nc.vector.tensor_tensor(
                                out=d1[:], in0=qparts[j][:], in1=crev[:],
                                op=SUB)
                            s1 = pa.tile([128, 512], f32, tag="s1", bufs=2)
                            nc.vector.tensor_scalar(
                                out=s1[:], in0=d1[:], scalar1=sel[:, 0:1],
                                scalar2=None, op0=MUL)
                            nc.vector.tensor_tensor(
                                out=x_r[db][:, Lc + jl:Lc + jl + 512],
                                in0=crev[:], in1=s1[:], op=ADD)

                # ---------- phases B/C/D ----------
                with (tc.tile_pool(name="pb", bufs=1) as pb,
                      tc.tile_pool(name="psxp", bufs=1, space="PSUM") as psxp):
                    xp_acc = [psxp.tile([96, 512], f32, tag=f"xp{tb}",
                                        name=f"xp{tb}") for tb in range(NT)]
                    for e in range(NE):
                        wt = pb.tile([128, NK * 128], f16, tag="winstream",
                                     bufs=2)
                        nc.gpsimd.dma_start(wt[:], Win_d[e, :, :])
                        upre = pb.tile([128, L + 3], f32, tag="upre", bufs=2)
                        nc.gpsimd.memset(upre[:, 0:3], 0.0)
                        for tb in range(NT):
                            acc = ps.tile([128, 512], f32, tag="pp")
                            for kb in range(NK):
                                nc.tensor.matmul(
                                    acc[:], wt[:, kb * 128:(kb + 1) * 128],
                                    x_r[kb][:, tb * 512:(tb + 1) * 512],
                                    start=(kb == 0), stop=(kb == NK - 1))
                            nc.scalar.copy(
                                upre[:, 3 + tb * 512: 3 + (tb + 1) * 512],
                                acc[:])
                        # causal depthwise conv: taps read aligned slices
                        cacc = pb.tile([128, L], f32, tag="cacc0", bufs=2)
                        nc.vector.tensor_scalar(
                            out=cacc[:], in0=upre[:, 0:L],
                            scalar1=convw[:, e * DCONV: e * DCONV + 1],
                            scalar2=None, op0=MUL)
                        for k in (1, 2, 3):
                            nxt = pb.tile([128, L], f32, tag=f"cacc{k % 2}",
                                          name=f"cacc_{k}", bufs=2)
                            nc.vector.scalar_tensor_tensor(
                                out=nxt[:], in0=upre[:, k:k + L],
                                scalar=convw[:, e * DCONV + k:
                                             e * DCONV + k + 1],
                                in1=cacc[:], op0=MUL, op1=ADD)
                            cacc = nxt
                        usilu = pb.tile([128, L], f16, tag="usilu", bufs=2)
                        nc.scalar.activation(usilu[:], cacc[:], AF.Silu,
                                             bias=convb[:, e:e + 1])
                        nc.gpsimd.dma_start(
                            u_sp[e * 128:(e + 1) * 128, :], usilu[:])
                        for tb in range(NT):
                            nc.tensor.matmul(
                                xp_acc[tb][:],
                                Wxp[:, e * 96:(e + 1) * 96],
                                usilu[:, tb * 512:(tb + 1) * 512],
                                start=(e == 0), stop=(e == NE - 1))

                    # phase C: z half -> silu -> spill
                    for e in range(NE):
                        wt = pb.tile([128, NK * 128], f16, tag="winstream",
                                     name="wtz", bufs=2)
                        nc.gpsimd.dma_start(wt[:], Win_d[NE + e, :, :])
                        for tb in range(NT):
                            acc = ps.tile([128, 512], f32, tag="pp")
                            for kb in range(NK):
                                nc.tensor.matmul(
                                    acc[:], wt[:, kb * 128:(kb + 1) * 128],
                                    x_r[kb][:, tb * 512:(tb + 1) * 512],
                                    start=(kb == 0), stop=(kb == NK - 1))
                            zt = pb.tile([128, 512], bf16, tag="zt", bufs=2)
                            nc.scalar.activation(zt[:], acc[:], AF.Silu)
                            nc.sync.dma_start(
                                zs_sp[e * 128:(e + 1) * 128,
                                      tb * 512:(tb + 1) * 512], zt[:])

                    # phase D: x_proj epilogue
                    for tb in range(NT):
                        nc.scalar.copy(dt_r[:, tb * 512:(tb + 1) * 512],
                                       xp_acc[tb][0:DTR, :])
                        bct = pb.tile([2 * DS, 512], bf16, tag="bct", bufs=2)
                        nc.scalar.copy(bct[:], xp_acc[tb][DTR:96, :])
                        nc.sync.dma_start(
                            bc_sp[:, tb * 512:(tb + 1) * 512], bct[:])

            # ---------- phase E: dt_proj -> delta, dg ----------
            with tc.tile_pool(name="pe", bufs=1) as pe:
                for e in range(NE):
                    delta = pe.tile([128, L], f32, tag="delta", bufs=2)
                    for tb in range(NT):
                        acc = ps.tile([128, 512], f32, tag="pp")
                        nc.tensor.matmul(
                            acc[:], Wdt[:, e * 128:(e + 1) * 128],
                            dt_r[:, tb * 512:(tb + 1) * 512],
                            start=True, stop=True)
                        # softplus(x + b) = ln(1 + exp(x + b)); inputs here
                        # are small (|x|<6) so exp cannot overflow
                        ex = pe.tile([128, 512], f32, tag="spexp", bufs=2)
                        nc.scalar.activation(
                            ex[:], acc[:], AF.Exp, bias=dtb[:, e:e + 1])
                        nc.scalar.activation(
                            delta[:, tb * 512:(tb + 1) * 512], ex[:],
                            AF.Ln, bias=1.0)
                    nc.gpsimd.dma_start(
                        dl_sp[e * 128:(e + 1) * 128, :], delta[:])
                    ub = pe.tile([128, L], f16, tag="ub_e", bufs=2)
                    nc.sync.dma_start(ub[:], u_sp[e * 128:(e + 1) * 128, :])
                    dg = pe.tile([128, L], f16, tag="dg_e", bufs=2)
                    nc.vector.tensor_tensor(out=dg[:], in0=delta[:],
                                            in1=ub[:], op=MUL)
                    nc.sync.dma_start(
                        dg_sp[e * 128:(e + 1) * 128, :], dg[:])

            # ---------- phase F: selective scan ----------
            with tc.tile_pool(name="pf", bufs=1) as pf:
                for p in range(2):
                    Bb, Cb = [], []
                    for si in range(8):
                        s = p * 8 + si
                        bb = pf.tile([128, L], bf16, tag=f"Bb{si}",
                                     name=f"Bb{si}")
                        nc.sync.dma_start(
                            bb[:], bc_sp[s:s + 1, :].partition_broadcast(128))
                        cb = pf.tile([128, L], bf16, tag=f"Cb{si}",
                                     name=f"Cb{si}")
                        nc.sync.dma_start(
                            cb[:],
                            bc_sp[DS + s:DS + s + 1, :].partition_broadcast(128))
                        Bb.append(bb)
                        Cb.append(cb)
                    for e in range(NE):
                        dl = pf.tile([128, L], f16, tag="dl_f", bufs=2)
                        nc.sync.dma_start(
                            dl[:], dl_sp[e * 128:(e + 1) * 128, :])
                        dgt = pf.tile([128, L], f16, tag="dg_f", bufs=2)
                        nc.sync.dma_start(
                            dgt[:], dg_sp[e * 128:(e + 1) * 128, :])
                        if p == 0:
                            ub = pf.tile([128, L], f16, tag="ub_f", bufs=2)
                            nc.sync.dma_start(
                                ub[:], u_sp[e * 128:(e + 1) * 128, :])
                            yacc = pf.tile([128, L], f32, tag="yacc0",
                                           name="yacc_i", bufs=1)
                            nc.vector.tensor_scalar(
                                out=yacc[:], in0=ub[:],
                                scalar1=Dh[:, e:e + 1], scalar2=None, op0=MUL)
                        else:
                            yacc = pf.tile([128, L], f32, tag="yacc0",
                                           name="yacc_l", bufs=1)
                            nc.sync.dma_start(
                                yacc[:], yacc_sp[e * 128:(e + 1) * 128, :])
                        for si in range(8):
                            s = p * 8 + si
                            dA = pf.tile([128, L], f32, tag="dA", bufs=2)
                            nc.scalar.activation(
                                dA[:], dl[:], AF.Exp,
                                scale=Ah[:, e * DS + s: e * DS + s + 1])
                            dgB = pf.tile([128, L], bf16, tag="dgB", bufs=2)
                            nc.vector.tensor_tensor(
                                out=dgB[:], in0=dgt[:], in1=Bb[si][:], op=MUL)
                            h = pf.tile([128, L], bf16, tag="h", bufs=2)
                            nc.vector.tensor_tensor_scan(
                                h[:], dA[:], dgB[:], 0.0, op0=MUL, op1=ADD)
                            ch = pf.tile([128, L], bf16, tag="ch", bufs=2)
                            nc.vector.tensor_tensor(
                                out=ch[:], in0=h[:], in1=Cb[si][:], op=MUL)
                            ynew = pf.tile([128, L], f32,
                                           tag=f"yacc{(si + 1) % 2}",
                                           name=f"yacc_{si}", bufs=1)
                            nc.gpsimd.tensor_tensor(
                                out=ynew[:], in0=yacc[:], in1=ch[:], op=ADD)
                            yacc = ynew
                        if p == 0:
                            nc.sync.dma_start(
                                yacc_sp[e * 128:(e + 1) * 128, :], yacc[:])
                        else:
                            zst = pf.tile([128, L], bf16, tag="zs_f", bufs=2)
                            nc.sync.dma_start(
                                zst[:], zs_sp[e * 128:(e + 1) * 128, :])
                            yg = pf.tile([128, L], f16, tag="yg", bufs=2)
                            nc.vector.tensor_tensor(
                                out=yg[:], in0=yacc[:], in1=zst[:], op=MUL)
                            nc.sync.dma_start(
                                yg_sp[e * 128:(e + 1) * 128, :], yg[:])

            # ---------- phase G: out_proj on the sel-chosen half ----------
            with tc.tile_pool(name="pg", bufs=1) as pg:
                Wout = pg.tile([128, NE * DM], f16, tag="Wout")
                nc.gpsimd.dma_start(Wout[:], Wout_d[:])
                for j in range(2):
                    jl = j * 512
                    ysel = []
                    for kb in range(NE):
                        # fwd (sel=1): natural cols Lc+jl..; bwd (sel=0):
                        # cols (1-j)*512.. time-reversed
                        ylo = pg.tile([128, 512], f16, tag="ylo", bufs=2)
                        nc.sync.dma_start(
                            ylo[:], yg_sp[kb * 128:(kb + 1) * 128,
                                          (1 - j) * 512:(2 - j) * 512])
                        yhi = pg.tile([128, 512], f16, tag="yhi", bufs=2)
                        nc.sync.dma_start(
                            yhi[:], yg_sp[kb * 128:(kb + 1) * 128,
                                          Lc + jl:Lc + jl + 512])
                        yrev = pg.tile([128, 512], f16, tag="yrev", bufs=2)
                        nc.scalar.copy(yrev[:], ylo[:, ::-1])
                        dft = pg.tile([128, 512], f32, tag="dft", bufs=2)
                        nc.vector.tensor_tensor(
                            out=dft[:], in0=yhi[:], in1=yrev[:], op=SUB)
                        sdf = pg.tile([128, 512], f32, tag="sdf", bufs=2)
                        nc.vector.tensor_scalar(
                            out=sdf[:], in0=dft[:], scalar1=sel[:, 0:1],
                            scalar2=None, op0=MUL)
                        ys = pg.tile([128, 512], f16, tag=f"ys{kb}",
                                     name=f"ys{kb}", bufs=1)
                        nc.vector.tensor_tensor(
                            out=ys[:], in0=yrev[:], in1=sdf[:], op=ADD)
                        ysel.append(ys)
                    for mb in range(8):
                        acc = ps.tile([128, 512], f32, tag="pp")
                        for kb in range(NE):
                            nc.tensor.matmul(
                                acc[:],
                                Wout[:, kb * DM + mb * 128:
                                     kb * DM + (mb + 1) * 128],
                                ysel[kb][:], start=(kb == 0),
                                stop=(kb == NE - 1))
                        ot = pg.tile([128, 512], f16, tag="ot", bufs=2)
                        nc.scalar.copy(ot[:], acc[:])
                        nc.sync.dma_start(
                            og_sp[mb * 128:(mb + 1) * 128, jl:jl + 512],
                            ot[:])

                # pair AllReduce: fwd+bwd sum lands on both cores
                nc.gpsimd.collective_compute(
                    "AllReduce", ADD, replica_groups=PAIRS,
                    ins=[og_sp[:].opt()], outs=[og_sum[:].opt()])

                # each core outputs its row half: fwd rows 0:512, bwd 512:1024
                for bb in range(4):
                    for j in range(2):
                        jl = j * 512
                        stp = pg.tile([128, 512], f16, tag="stp", bufs=2)
                        nc.sync.dma_start(
                            stp[:], og_sum[bb * 128:(bb + 1) * 128,
                                           jl:jl + 512])
                        sbt = pg.tile([128, 512], f16, tag="sbt", bufs=2)
                        nc.sync.dma_start(
                            sbt[:], og_sum[(4 + bb) * 128:(5 + bb) * 128,
                                           jl:jl + 512])
                        dfo = pg.tile([128, 512], f32, tag="dfo", bufs=2)
                        nc.vector.tensor_tensor(
                            out=dfo[:], in0=stp[:], in1=sbt[:], op=SUB)
                        sfo = pg.tile([128, 512], f32, tag="sfo", bufs=2)
                        nc.vector.tensor_scalar(
                            out=sfo[:], in0=dfo[:], scalar1=sel[:, 0:1],
                            scalar2=None, op0=MUL)
                        oh = pg.tile([128, 512], f16, tag="oh", bufs=2)
                        nc.vector.tensor_tensor(
                            out=oh[:], in0=sbt[:], in1=sfo[:], op=ADD)
                        nc.sync.dma_start(
                            out_d[bb * 128:(bb + 1) * 128, jl:jl + 512],
                            oh[:])

    nc.compile()
    return nc


def _install_cc_cache():
    """Content-keyed disk cache around the neuronx compiler hook.

    The bass_exec compile path (walrus) takes ~60 s for this program and
    has no persistent cache of its own; the emitted BIR (and hence the
    HLO carrying it) is byte-deterministic, so a sha256-of-HLO keyed
    cache makes every process after the first skip the compile.
    """
    import os
    try:
        import libneuronxla
    except ImportError:
        return
    if getattr(libneuronxla, "_bass_cc_disk_cache", False):
        return
    inner = libneuronxla.neuronx_cc
    cache_dir = os.environ.get(
        "NEURON_COMPILE_CACHE_URL",
        os.path.join(os.path.expanduser("~"), ".neuron-compile-cache"))
    try:
        os.makedirs(cache_dir, exist_ok=True)
    except OSError:
        libneuronxla._bass_cc_disk_cache = True
        return

    def cached(code, code_format, platform_version, file_prefix,
               *a, **kw):
        c = code if isinstance(code, (bytes, bytearray)) else \
            str(code).encode()
        key = hashlib.sha256(
            c + b"|" + str(platform_version).encode()).hexdigest()
        path = os.path.join(cache_dir, f"bassneff-{key}.hlo")
        try:
            with open(path, "rb") as f:
                return 0, f.read()
        except OSError:
            pass
        r = inner(code, code_format, platform_version, file_prefix,
                  *a, **kw)
        try:
            err, blob = r
            if err == 0 and isinstance(blob, (bytes, bytearray)) and blob:
                tmp = f"{path}.tmp.{os.getpid()}"
                with open(tmp, "wb") as f:
                    f.write(blob)
                os.replace(tmp, path)
        except Exception:
            pass
        return r

    libneuronxla.neuronx_cc = cached
    libneuronxla._bass_cc_disk_cache = True


def _make_runner(nc):
    """Jit the SPMD dispatch once; repeat calls hit the C++ fast path.

    Mirrors what bass_utils.run_bass_kernel_spmd does under axon
    (bass2jax.run_bass_via_pjrt), minus the per-call re-jit and the
    donated zero output buffers (the kernel writes every output
    element, so uninitialized outputs are fine).
    """
    import jax
    import numpy as np
    from jax.sharding import Mesh, PartitionSpec
    from jax.experimental.shard_map import shard_map
    from concourse import mybir
    from concourse.bass2jax import (_bass_exec_p, install_neuronx_cc_hook,
                                    partition_id_tensor)

    install_neuronx_cc_hook()
    _install_cc_cache()
    partition_name = (nc.partition_id_tensor.name
                      if nc.partition_id_tensor else None)
    in_names, out_names, out_avals = [], [], []
    for alloc in nc.m.functions[0].allocations:
        if not isinstance(alloc, mybir.MemoryLocationSet):
            continue
        name = alloc.memorylocations[0].name
        if alloc.kind == "ExternalInput":
            if name != partition_name:
                in_names.append(name)
        elif alloc.kind == "ExternalOutput":
            out_names.append(name)
            out_avals.append(jax.core.ShapedArray(
                tuple(alloc.tensor_shape), mybir.dt.np(alloc.dtype)))
    in_names_all = list(in_names)
    if partition_name is not None:
        in_names_all.append(partition_name)

    def _body(*args):
        operands = list(args)
        if partition_name is not None:
            operands.append(partition_id_tensor())
        return tuple(_bass_exec_p.bind(
            *operands, out_avals=tuple(out_avals),
            in_names=tuple(in_names_all), out_names=tuple(out_names),
            lowering_input_output_aliases=(),
            sim_require_finite=True, sim_require_nnan=True, nc=nc))

    devices = jax.devices()[:NCORE]
    mesh = Mesh(np.asarray(devices), ("core",))
    n_in = len(in_names)
    sharded = jax.jit(shard_map(
        _body, mesh=mesh, in_specs=(PartitionSpec("core"),) * n_in,
        out_specs=(PartitionSpec("core"),) * len(out_names),
        check_rep=False))

    def run(per_name_globals):
        args = [per_name_globals[name] for name in in_names]
        outs = sharded(*args)
        return {name: outs[i] for i, name in enumerate(out_names)}

    return run


def kernel(**inputs) -> np.ndarray:
    global _RUN, _WFP
    fp = _fingerprint(inputs)
    if _RUN is None or fp != _WFP:
        _RUN = _make_runner(_build(_prep_weights(inputs)))
        _WFP = fp

    q16 = np.asarray(inputs["query"], np.float32).astype(np.float16)
    c16 = np.asarray(inputs["context"], np.float32).astype(np.float16)
    gact = np.empty((NCORE, NA * 128, 512), np.float16)
    for b in range(B):
        for hf in range(2):
            hs = slice(hf * 512, (hf + 1) * 512)
            core = b + 4 * hf
            gact[core, :DC] = c16[b, hs].T
            gact[core, DC:] = q16[b, hs].T
    gsel = np.zeros((NCORE, 1), np.float32)
    gsel[:4] = 1.0

    res = _RUN({"acth": gact.reshape(NCORE * NA * 128, 512), "selv": gsel})
    o = np.asarray(res["out"]).reshape(NCORE, DM // 2, Lq)
    y = np.empty((B, Lq, DM), np.float32)
    half = np.empty((DM, Lq), np.float32)
    for b in range(B):
        half[:DM // 2] = o[b]
        half[DM // 2:] = o[4 + b]
        np.multiply(half.T, 0.5, out=y[b])
    return y
